# revision 28
# baseline (speedup 1.0000x reference)
"""Distributed Trainium2 kernel for nn_ADQC_basic (23-qubit state, 33 two-qubit
gates, 3-layer brickwall).

Strategy: state-vector qubit partitioning over 8 cores (shard qubits q20..q22),
SBUF-resident bf16 re/im planes [128 partitions x 8192]. Gates are fused into
six 7-qubit "window" unitaries applied as dense 128x128 matmuls on the
partition dim (4 real matmuls per complex product, PSUM-accumulated). Qubits
rotate through the partition dim via DMA x-bar transposes (partition <-> low-7
free bits); free-bit reorders are folded into the PSUM->SBUF evacuation copies.
One AllToAll mid-kernel re-shards q20..q22 -> q0..q2 so the last window can
touch the top qubits.
"""
import numpy as np
import ml_dtypes

N_QUBIT = 23

# ---------------------------------------------------------------------------
# schedule (validated against the jax reference by the numpy mirror)
# ---------------------------------------------------------------------------
# (partition bit order msb-first, gates [(layer, qa, qb), ...] in application order)
WINDOWS = [
    ([0, 1, 2, 3, 4, 5, 6],        [(0,0,1),(0,2,3),(0,4,5),(1,1,2),(1,3,4),(2,0,1),(2,2,3)]),
    ([7, 8, 9, 10, 11, 12, 13],    [(0,8,9),(0,10,11),(0,12,13),(1,9,10),(1,11,12),(2,10,11)]),
    ([0, 17, 18, 19, 16, 14, 15],  [(0,14,15),(0,16,17),(0,18,19),(1,15,16),(1,17,18),(2,16,17)]),
    ([1, 4, 5, 6, 7, 8, 9],        [(0,6,7),(1,5,6),(1,7,8),(2,4,5),(2,6,7),(2,8,9)]),
    ([12, 13, 18, 19, 16, 14, 15], [(1,13,14),(2,12,13),(2,14,15)]),
    ([20, 21, 22, 12, 13, 18, 19], [(0,20,21),(1,19,20),(1,21,22),(2,18,19),(2,20,21)]),
]

# free-bit layouts (msb-first, 13 bits) before each reorder, and reorder targets
REORDERS = [
    # (src_layout, dst_layout) applied in the PSUM->SBUF copies of window k
    (list(range(7, 20)),
     [14, 15, 16, 17, 18, 19, 7, 8, 9, 10, 11, 12, 13]),
    ([14, 15, 16, 17, 18, 19, 0, 1, 2, 3, 4, 5, 6],
     [1, 2, 3, 4, 5, 6, 0, 17, 18, 19, 16, 14, 15]),
    ([1, 2, 3, 4, 5, 6, 7, 8, 9, 10, 11, 12, 13],
     [2, 3, 10, 11, 12, 13, 1, 4, 5, 6, 7, 8, 9]),
    ([2, 3, 10, 11, 12, 13, 0, 17, 18, 19, 16, 14, 15],
     [2, 3, 10, 11, 0, 17, 12, 13, 18, 19, 16, 14, 15]),
    ([2, 3, 10, 11, 0, 17, 1, 4, 5, 6, 7, 8, 9],
     [0, 1, 2, 3, 10, 11, 4, 5, 6, 7, 8, 9, 17]),
    # V6: identity (final layout)
    ([16, 14, 15, 3, 10, 11, 4, 5, 6, 7, 8, 9, 17],
     [16, 14, 15, 3, 10, 11, 4, 5, 6, 7, 8, 9, 17]),
]

FINAL_BITS = [20, 21, 22, 12, 13, 18, 19] + REORDERS[5][1]  # per-core local order


def gate_index(layer, qa):
    if layer == 0:
        return qa // 2
    if layer == 1:
        return 11 + (qa - 1) // 2
    return 22 + qa // 2


def polar_unitaries(gate_paras):
    gs = np.asarray(gate_paras).astype(np.complex128)
    us = []
    for g in gs:
        u, _, vh = np.linalg.svd(g)
        us.append(u @ vh)
    return us


def window_matrix(pbits, gates, us):
    W = np.eye(128, dtype=np.complex128)
    for (layer, qa, qb) in gates:
        u = us[gate_index(layer, qa)]
        ia, ib = pbits.index(qa), pbits.index(qb)
        sa, sb = 1 << (6 - ia), 1 << (6 - ib)
        G = np.zeros((128, 128), dtype=np.complex128)
        for p in range(128):
            a = (p // sa) & 1
            b = (p // sb) & 1
            base = p - a * sa - b * sb
            for a2 in range(2):
                for b2 in range(2):
                    G[base + a2 * sa + b2 * sb, p] = u[a2 * 2 + b2, a * 2 + b]
        W = G @ W
    return W


# ---------------------------------------------------------------------------
# reorder-copy planner: PSUM chunk [128, 512] -> strided SBUF writes
# ---------------------------------------------------------------------------
def plan_reorder(src_order, dst_order, nbits=13, chunk_bits=4, max_free_dims=3):
    """Returns per-chunk sub-copy list:
    [ (in_off, out_off, [(in_step, out_step, count), ...outer->inner]) ... ]
    chunks iterate the top `chunk_bits` of src_order."""
    W = src_order[chunk_bits:]
    nW = len(W)
    src_pos = {b: (nW - 1 - i) for i, b in enumerate(W)}          # power within chunk
    dst_pos = {b: (nbits - 1 - i) for i, b in enumerate(dst_order)}
    bits = sorted(W, key=lambda b: -dst_pos[b])
    fields = [[bits[0]]]
    for b in bits[1:]:
        pb = fields[-1][-1]
        if dst_pos[pb] == dst_pos[b] + 1 and src_pos[pb] == src_pos[b] + 1:
            fields[-1].append(b)
        else:
            fields.append([b])
    fdims = [(1 << src_pos[f[-1]], 1 << dst_pos[f[-1]], 1 << len(f)) for f in fields]
    # split outermost fields until <= max_free_dims
    subs = [(0, 0)]
    while len(fdims) > max_free_dims:
        s_in, s_out, cnt = fdims.pop(0)
        subs = [(bi + v * s_in, bo + v * s_out) for (bi, bo) in subs for v in range(cnt)]
    plans = []
    n_chunks = 1 << chunk_bits
    for ci in range(n_chunks):
        out_off = 0
        for i in range(chunk_bits):
            if (ci >> (chunk_bits - 1 - i)) & 1:
                out_off += 1 << dst_pos[src_order[i]]
        plans.append([(bi, out_off + bo, list(fdims)) for (bi, bo) in subs])
    return plans


# ---------------------------------------------------------------------------
# bass kernel builder
# ---------------------------------------------------------------------------
def build_bass():
    import concourse.bass as bass
    import concourse.mybir as mybir
    from concourse import tile, bacc
    from concourse.bass import AP

    BF16 = mybir.dt.bfloat16
    F32 = mybir.dt.float32
    F = 8192
    CH = 512                 # psum chunk columns
    NCH = F // CH            # 16
    NCORES = 8

    nc = bacc.Bacc(None, target_bir_lowering=False, num_devices=NCORES)

    xr_ext = nc.declare_dram_parameter("xr", [128, F], BF16, isOutput=False)
    xi_ext = nc.declare_dram_parameter("xi", [128, F], BF16, isOutput=False)
    wm_ext = nc.declare_dram_parameter("wm", [128, 18 * 128], BF16, isOutput=False)
    out_ext = nc.declare_dram_parameter("out", [2, 128, F], BF16, isOutput=True)

    # two half-buffers (split by q3, the content-free MSB) so the AllToAll can
    # overlap the tail of window 4 and the head of window 5
    a2a_in_h = [nc.dram_tensor(f"a2a_in{h}", [NCORES, 2, 128, 512], BF16) for h in range(2)]
    a2a_out_h = [nc.dram_tensor(f"a2a_out{h}", [NCORES, 2, 128, 512], BF16) for h in range(2)]

    plans = [plan_reorder(s, d) for (s, d) in REORDERS[:5]]

    with tile.TileContext(nc) as tc:
        with (
            tc.tile_pool(name="state", bufs=1) as state_pool,
            tc.tile_pool(name="wpool", bufs=1) as wpool,
            tc.tile_pool(name="fin", bufs=1) as fin_pool,
            tc.tile_pool(name="psum", bufs=1, space="PSUM") as psum_pool,
        ):
            Ar = state_pool.tile([128, F], BF16, tag="Ar")
            Ai = state_pool.tile([128, F], BF16, tag="Ai")
            Br = state_pool.tile([128, F], BF16, tag="Br")
            Bi = state_pool.tile([128, F], BF16, tag="Bi")
            wsb = wpool.tile([128, 18 * 128], BF16, tag="wsb")

            nc.sync.dma_start(out=wsb[:, :], in_=wm_ext[:, :])
            for q in range(4):
                qs = slice(q * (F // 4), (q + 1) * (F // 4))
                nc.sync.dma_start(out=Ar[:, qs], in_=xr_ext[:, qs])
                nc.sync.dma_start(out=Ai[:, qs], in_=xi_ext[:, qs])

            def wmat(k, j):  # stationary lhsT slice for window k variant j
                c0 = (k * 3 + j) * 128
                return wsb[:, c0:c0 + 128]

            def sbuf_ap(t, off, dims):
                base = t[:, :]
                ap = [list(base.ap[0])] + [[s, c] for (s, c) in dims]
                return AP(base.tensor, base.offset + off, ap)

            eng_load = [0.0, 0.0]  # running DVE / ACT copy-cost estimate

            def run_window(k, src_r, src_i, dst_r, dst_i, out_f32=None, round_done=None,
                           chunk_order=None, src_of=None):
                plan = plans[k] if k < 5 else None
                if chunk_order is None:
                    chunk_order = list(range(NCH))
                if src_of is None:
                    src_of = lambda plane, c: (src_r if plane == 0 else src_i,
                                               slice(c * CH, (c + 1) * CH))
                for rnd in range(4):
                    ps_r, ps_i = [], []
                    for cc in range(4):
                        c = chunk_order[rnd * 4 + cc]
                        sl = slice(c * CH, (c + 1) * CH)
                        pr = psum_pool.tile([128, CH], F32, tag=f"pr{cc}")
                        pi = psum_pool.tile([128, CH], F32, tag=f"pi{cc}")
                        ps_r.append((pr, c, sl))
                        ps_i.append((pi, c, sl))
                    # stationary-major: Wi -> psI start ; Wr -> psI stop, psR start ; -Wi -> psR stop
                    for (pi, c, sl) in ps_i:
                        t, s = src_of(0, c)
                        nc.tensor.matmul(pi[:, :], wmat(k, 2), t[:, s], start=True, stop=False)
                    for (pi, c, sl) in ps_i:
                        t, s = src_of(1, c)
                        nc.tensor.matmul(pi[:, :], wmat(k, 0), t[:, s], start=False, stop=True)
                    for (pr, c, sl) in ps_r:
                        t, s = src_of(0, c)
                        nc.tensor.matmul(pr[:, :], wmat(k, 0), t[:, s], start=True, stop=False)
                    for (pr, c, sl) in ps_r:
                        t, s = src_of(1, c)
                        nc.tensor.matmul(pr[:, :], wmat(k, 1), t[:, s], start=False, stop=True)
                    # evacuate — balance copies across DVE (0.96 GHz) and
                    # ACT (1.2 GHz) by estimated cost instead of by plane
                    def do_copy(out_ap, in_ap, elems):
                        dve_t = (elems + 140) / 0.96
                        act_t = (elems + 172) / 1.2
                        if eng_load[0] + dve_t <= eng_load[1] + act_t:
                            eng_load[0] += dve_t
                            nc.vector.tensor_copy(out_ap, in_ap)
                        else:
                            eng_load[1] += act_t
                            nc.scalar.copy(out_ap, in_ap)

                    for (plist, dst, plane) in ((ps_r, dst_r, 0), (ps_i, dst_i, 1)):
                        for (pt, c, sl) in plist:
                            if out_f32 is not None:
                                do_copy(out_f32[plane][:, sl], pt[:, :], CH)
                            elif plan is None:
                                do_copy(dst[:, sl], pt[:, :], CH)
                            else:
                                for (in_off, out_off, dims) in plan[c]:
                                    n_el = 1
                                    for (_, _, cnt) in dims:
                                        n_el *= cnt
                                    in_ap = sbuf_ap(pt, in_off, [(si, cnt) for (si, so, cnt) in dims])
                                    out_ap = sbuf_ap(dst, out_off, [(so, cnt) for (si, so, cnt) in dims])
                                    do_copy(out_ap, in_ap, n_el)
                    if round_done is not None:
                        round_done(rnd)

            def xbar(dst, src, order=None):
                # swap partition(7) <-> free-low-7, split into column eighths:
                # queue parallelism on HW, and the next window's first rounds
                # only depend on the early slices. `order` matches the slice
                # emission to the consumer's chunk order.
                for h in (order or range(8)):
                    sl = slice(h * 1024, (h + 1) * 1024)
                    out3 = dst[:, sl].rearrange("p (t j) -> p t j", t=8, j=128)
                    nc.sync.dma_start(out=out3, in_=src[:, sl], transpose=True)

            # windows 0..4 with transposes; split A2A; window 5
            cur_r, cur_i, oth_r, oth_i = Ar, Ai, Br, Bi
            for k in range(4):
                run_window(k, cur_r, cur_i, oth_r, oth_i)
                # before window 4 (q3-ordered chunks 0-3, 8-11, 4-7, 12-15),
                # emit the slices feeding its early rounds first
                t_order = [0, 1, 4, 5, 2, 3, 6, 7] if k == 3 else None
                xbar(cur_r, oth_r, order=t_order)
                xbar(cur_i, oth_i, order=t_order)

            def emit_half(h):
                # blocks of half h are complete once V5's q3==h chunks copied
                for cp in range(NCORES):
                    sl = slice(cp * 1024 + h * 512, cp * 1024 + (h + 1) * 512)
                    nc.sync.dma_start(out=a2a_in_h[h][cp, 0], in_=oth_r[:, sl])
                    nc.sync.dma_start(out=a2a_in_h[h][cp, 1], in_=oth_i[:, sl])
                nc.gpsimd.collective_compute(
                    "AllToAll",
                    mybir.AluOpType.bypass,
                    replica_groups=[list(range(NCORES))],
                    ins=[a2a_in_h[h].ap().opt()],
                    outs=[a2a_out_h[h].ap().opt()],
                )

            # V5 source layout L_5 = [2,3,10,11,...]: chunk bit q3 has value 4
            v5_order = [c for c in range(NCH) if not (c & 4)] + \
                       [c for c in range(NCH) if (c & 4)]
            run_window(4, cur_r, cur_i, oth_r, oth_i,
                       chunk_order=v5_order,
                       round_done=lambda rnd: emit_half(rnd // 2) if rnd in (1, 3) else None)

            # compacted half tiles for window 5's input (keeps deps half-granular)
            G0r = fin_pool.tile([128, F // 2], BF16, tag="G0r")
            G0i = fin_pool.tile([128, F // 2], BF16, tag="G0i")
            G1r = fin_pool.tile([128, F // 2], BF16, tag="G1r")
            G1i = fin_pool.tile([128, F // 2], BF16, tag="G1i")
            G = [[G0r, G0i], [G1r, G1i]]
            for h in range(2):
                base = a2a_out_h[h].ap()
                for plane in range(2):
                    # [8, 2, 128, 512]: partition=(s, p_hi4), free=(p_lo3, f)
                    src = AP(base.tensor, base.offset + plane * 65536,
                             [[131072, 8], [4096, 16], [1, 4096]])
                    nc.sync.dma_start(out=G[h][plane][:, :], in_=src)

            # window 5 reads compacted halves: old chunk c -> G[c&1] col-block c>>1
            def v6_src(plane, c):
                return G[c & 1][plane], slice((c >> 1) * CH, ((c >> 1) + 1) * CH)

            v6_order = [c for c in range(NCH) if not (c & 1)] + \
                       [c for c in range(NCH) if (c & 1)]
            Fr = fin_pool.tile([128, F], BF16, tag="Fr")
            Fi = fin_pool.tile([128, F], BF16, tag="Fi")

            def v6_round_done(rnd):
                for cc in range(4):
                    c = v6_order[rnd * 4 + cc]
                    sl = slice(c * CH, (c + 1) * CH)
                    nc.sync.dma_start(out=out_ext[0, :, sl], in_=Fr[:, sl])
                    nc.sync.dma_start(out=out_ext[1, :, sl], in_=Fi[:, sl])

            run_window(5, None, None, None, None, out_f32=(Fr, Fi),
                       chunk_order=v6_order, src_of=v6_src,
                       round_done=v6_round_done)

    nc.compile()
    return nc


# ---------------------------------------------------------------------------
# host entry
# ---------------------------------------------------------------------------
_CACHED = {}


def _get_nc():
    if "nc" not in _CACHED:
        _CACHED["nc"] = build_bass()
    return _CACHED["nc"]


def kernel(state, gate_paras):
    from concourse.bass_utils import run_bass_kernel_spmd

    state = np.asarray(state)
    gate_paras = np.asarray(gate_paras)
    us = polar_unitaries(gate_paras)

    wmflat = np.zeros((128, 18 * 128), dtype=np.float64)
    for k, (pbits, gates) in enumerate(WINDOWS):
        W = window_matrix(pbits, gates, us)
        Wr, Wi = W.real, W.imag
        wmflat[:, (k * 3 + 0) * 128:(k * 3 + 1) * 128] = Wr.T
        wmflat[:, (k * 3 + 1) * 128:(k * 3 + 2) * 128] = -Wi.T
        wmflat[:, (k * 3 + 2) * 128:(k * 3 + 3) * 128] = Wi.T
    wm_bf = wmflat.astype(ml_dtypes.bfloat16)

    full = state.reshape(-1)
    in_maps = []
    for c in range(8):
        loc = full[c::8]
        in_maps.append({
            "xr": loc.real.astype(ml_dtypes.bfloat16).reshape(128, 8192),
            "xi": loc.imag.astype(ml_dtypes.bfloat16).reshape(128, 8192),
            "wm": wm_bf,
        })

    nc = _get_nc()
    res = run_bass_kernel_spmd(nc, in_maps, core_ids=list(range(8)))
    results = res.results

    out = np.zeros((2,) * N_QUBIT, dtype=np.complex64)
    natural = list(range(3, 23))
    perm = [FINAL_BITS.index(q) for q in natural]
    for c in range(8):
        o = results[c]["out"]
        arr = (o[0].astype(np.float32) + 1j * o[1].astype(np.float32)).astype(np.complex64)
        arr = arr.reshape((2,) * 20).transpose(perm)
        out[(c >> 2) & 1, (c >> 1) & 1, c & 1] = arr
    return out


# revision 29
# speedup vs baseline: 1.0051x; 1.0051x over previous
"""Distributed Trainium2 kernel for nn_ADQC_basic (23-qubit state, 33 two-qubit
gates, 3-layer brickwall).

Strategy: state-vector qubit partitioning over 8 cores (shard qubits q20..q22),
SBUF-resident bf16 re/im planes [128 partitions x 8192]. Gates are fused into
six 7-qubit "window" unitaries applied as dense 128x128 matmuls on the
partition dim (4 real matmuls per complex product, PSUM-accumulated). Qubits
rotate through the partition dim via DMA x-bar transposes (partition <-> low-7
free bits); free-bit reorders are folded into the PSUM->SBUF evacuation copies.
One AllToAll mid-kernel re-shards q20..q22 -> q0..q2 so the last window can
touch the top qubits.
"""
import numpy as np
import ml_dtypes

N_QUBIT = 23

# ---------------------------------------------------------------------------
# schedule (validated against the jax reference by the numpy mirror)
# ---------------------------------------------------------------------------
# (partition bit order msb-first, gates [(layer, qa, qb), ...] in application order)
WINDOWS = [
    ([0, 1, 2, 3, 4, 5, 6],        [(0,0,1),(0,2,3),(0,4,5),(1,1,2),(1,3,4),(2,0,1),(2,2,3)]),
    ([7, 8, 9, 10, 11, 12, 13],    [(0,8,9),(0,10,11),(0,12,13),(1,9,10),(1,11,12),(2,10,11)]),
    ([0, 17, 18, 19, 16, 14, 15],  [(0,14,15),(0,16,17),(0,18,19),(1,15,16),(1,17,18),(2,16,17)]),
    ([1, 4, 5, 6, 7, 8, 9],        [(0,6,7),(1,5,6),(1,7,8),(2,4,5),(2,6,7),(2,8,9)]),
    ([12, 13, 18, 19, 16, 14, 15], [(1,13,14),(2,12,13),(2,14,15)]),
    ([20, 21, 22, 12, 13, 18, 19], [(0,20,21),(1,19,20),(1,21,22),(2,18,19),(2,20,21)]),
]

# free-bit layouts (msb-first, 13 bits) before each reorder, and reorder targets
REORDERS = [
    # (src_layout, dst_layout) applied in the PSUM->SBUF copies of window k
    (list(range(7, 20)),
     [14, 15, 16, 17, 18, 19, 7, 8, 9, 10, 11, 12, 13]),
    ([14, 15, 16, 17, 18, 19, 0, 1, 2, 3, 4, 5, 6],
     [1, 2, 3, 4, 5, 6, 0, 17, 18, 19, 16, 14, 15]),
    ([1, 2, 3, 4, 5, 6, 7, 8, 9, 10, 11, 12, 13],
     [2, 3, 10, 11, 12, 13, 1, 4, 5, 6, 7, 8, 9]),
    ([2, 3, 10, 11, 12, 13, 0, 17, 18, 19, 16, 14, 15],
     [2, 3, 10, 11, 0, 17, 12, 13, 18, 19, 16, 14, 15]),
    ([2, 3, 10, 11, 0, 17, 1, 4, 5, 6, 7, 8, 9],
     [0, 1, 2, 3, 10, 11, 4, 5, 6, 7, 8, 9, 17]),
    # V6: identity (final layout)
    ([16, 14, 15, 3, 10, 11, 4, 5, 6, 7, 8, 9, 17],
     [16, 14, 15, 3, 10, 11, 4, 5, 6, 7, 8, 9, 17]),
]

FINAL_BITS = [20, 21, 22, 12, 13, 18, 19] + REORDERS[5][1]  # per-core local order


def gate_index(layer, qa):
    if layer == 0:
        return qa // 2
    if layer == 1:
        return 11 + (qa - 1) // 2
    return 22 + qa // 2


def polar_unitaries(gate_paras):
    gs = np.asarray(gate_paras).astype(np.complex128)
    us = []
    for g in gs:
        u, _, vh = np.linalg.svd(g)
        us.append(u @ vh)
    return us


def window_matrix(pbits, gates, us):
    W = np.eye(128, dtype=np.complex128)
    for (layer, qa, qb) in gates:
        u = us[gate_index(layer, qa)]
        ia, ib = pbits.index(qa), pbits.index(qb)
        sa, sb = 1 << (6 - ia), 1 << (6 - ib)
        G = np.zeros((128, 128), dtype=np.complex128)
        for p in range(128):
            a = (p // sa) & 1
            b = (p // sb) & 1
            base = p - a * sa - b * sb
            for a2 in range(2):
                for b2 in range(2):
                    G[base + a2 * sa + b2 * sb, p] = u[a2 * 2 + b2, a * 2 + b]
        W = G @ W
    return W


# ---------------------------------------------------------------------------
# reorder-copy planner: PSUM chunk [128, 512] -> strided SBUF writes
# ---------------------------------------------------------------------------
def plan_reorder(src_order, dst_order, nbits=13, chunk_bits=4, max_free_dims=3):
    """Returns per-chunk sub-copy list:
    [ (in_off, out_off, [(in_step, out_step, count), ...outer->inner]) ... ]
    chunks iterate the top `chunk_bits` of src_order."""
    W = src_order[chunk_bits:]
    nW = len(W)
    src_pos = {b: (nW - 1 - i) for i, b in enumerate(W)}          # power within chunk
    dst_pos = {b: (nbits - 1 - i) for i, b in enumerate(dst_order)}
    bits = sorted(W, key=lambda b: -dst_pos[b])
    fields = [[bits[0]]]
    for b in bits[1:]:
        pb = fields[-1][-1]
        if dst_pos[pb] == dst_pos[b] + 1 and src_pos[pb] == src_pos[b] + 1:
            fields[-1].append(b)
        else:
            fields.append([b])
    fdims = [(1 << src_pos[f[-1]], 1 << dst_pos[f[-1]], 1 << len(f)) for f in fields]
    # split outermost fields until <= max_free_dims
    subs = [(0, 0)]
    while len(fdims) > max_free_dims:
        s_in, s_out, cnt = fdims.pop(0)
        subs = [(bi + v * s_in, bo + v * s_out) for (bi, bo) in subs for v in range(cnt)]
    plans = []
    n_chunks = 1 << chunk_bits
    for ci in range(n_chunks):
        out_off = 0
        for i in range(chunk_bits):
            if (ci >> (chunk_bits - 1 - i)) & 1:
                out_off += 1 << dst_pos[src_order[i]]
        plans.append([(bi, out_off + bo, list(fdims)) for (bi, bo) in subs])
    return plans


# ---------------------------------------------------------------------------
# bass kernel builder
# ---------------------------------------------------------------------------
def build_bass():
    import concourse.bass as bass
    import concourse.mybir as mybir
    from concourse import tile, bacc
    from concourse.bass import AP

    BF16 = mybir.dt.bfloat16
    F32 = mybir.dt.float32
    F = 8192
    CH = 512                 # psum chunk columns
    NCH = F // CH            # 16
    NCORES = 8

    nc = bacc.Bacc(None, target_bir_lowering=False, num_devices=NCORES)

    xr_ext = nc.declare_dram_parameter("xr", [128, F], BF16, isOutput=False)
    xi_ext = nc.declare_dram_parameter("xi", [128, F], BF16, isOutput=False)
    wm_ext = nc.declare_dram_parameter("wm", [128, 18 * 128], BF16, isOutput=False)
    out_ext = nc.declare_dram_parameter("out", [2, 128, F], BF16, isOutput=True)

    # two half-buffers (split by q3, the content-free MSB) so the AllToAll can
    # overlap the tail of window 4 and the head of window 5
    a2a_in_h = [nc.dram_tensor(f"a2a_in{h}", [NCORES, 2, 128, 512], BF16) for h in range(2)]
    a2a_out_h = [nc.dram_tensor(f"a2a_out{h}", [NCORES, 2, 128, 512], BF16) for h in range(2)]

    plans = [plan_reorder(s, d) for (s, d) in REORDERS[:5]]

    with tile.TileContext(nc) as tc:
        with (
            tc.tile_pool(name="state", bufs=1) as state_pool,
            tc.tile_pool(name="wpool", bufs=1) as wpool,
            tc.tile_pool(name="fin", bufs=1) as fin_pool,
            tc.tile_pool(name="psum", bufs=1, space="PSUM") as psum_pool,
        ):
            Ar = state_pool.tile([128, F], BF16, tag="Ar")
            Ai = state_pool.tile([128, F], BF16, tag="Ai")
            Br = state_pool.tile([128, F], BF16, tag="Br")
            Bi = state_pool.tile([128, F], BF16, tag="Bi")
            wsb = wpool.tile([128, 18 * 128], BF16, tag="wsb")

            nc.sync.dma_start(out=wsb[:, :], in_=wm_ext[:, :])
            for q in range(4):
                qs = slice(q * (F // 4), (q + 1) * (F // 4))
                nc.sync.dma_start(out=Ar[:, qs], in_=xr_ext[:, qs])
                nc.sync.dma_start(out=Ai[:, qs], in_=xi_ext[:, qs])

            def wmat(k, j):  # stationary lhsT slice for window k variant j
                c0 = (k * 3 + j) * 128
                return wsb[:, c0:c0 + 128]

            def sbuf_ap(t, off, dims):
                base = t[:, :]
                ap = [list(base.ap[0])] + [[s, c] for (s, c) in dims]
                return AP(base.tensor, base.offset + off, ap)

            eng_load = [0.0, 0.0]  # running DVE / ACT copy-cost estimate

            def run_window(k, src_r, src_i, dst_r, dst_i, out_f32=None, round_done=None,
                           chunk_order=None, src_of=None):
                plan = plans[k] if k < 5 else None
                if chunk_order is None:
                    chunk_order = list(range(NCH))
                if src_of is None:
                    src_of = lambda plane, c: (src_r if plane == 0 else src_i,
                                               slice(c * CH, (c + 1) * CH))
                for rnd in range(4):
                    ps_r, ps_i = [], []
                    for cc in range(4):
                        c = chunk_order[rnd * 4 + cc]
                        sl = slice(c * CH, (c + 1) * CH)
                        pr = psum_pool.tile([128, CH], F32, tag=f"pr{cc}")
                        pi = psum_pool.tile([128, CH], F32, tag=f"pi{cc}")
                        ps_r.append((pr, c, sl))
                        ps_i.append((pi, c, sl))
                    # stationary-major: Wi -> psI start ; Wr -> psI stop, psR start ; -Wi -> psR stop
                    for (pi, c, sl) in ps_i:
                        t, s = src_of(0, c)
                        nc.tensor.matmul(pi[:, :], wmat(k, 2), t[:, s], start=True, stop=False)
                    for (pi, c, sl) in ps_i:
                        t, s = src_of(1, c)
                        nc.tensor.matmul(pi[:, :], wmat(k, 0), t[:, s], start=False, stop=True)
                    for (pr, c, sl) in ps_r:
                        t, s = src_of(0, c)
                        nc.tensor.matmul(pr[:, :], wmat(k, 0), t[:, s], start=True, stop=False)
                    for (pr, c, sl) in ps_r:
                        t, s = src_of(1, c)
                        nc.tensor.matmul(pr[:, :], wmat(k, 1), t[:, s], start=False, stop=True)
                    # evacuate — balance copies across DVE (0.96 GHz) and
                    # ACT (1.2 GHz) by estimated cost instead of by plane
                    def do_copy(out_ap, in_ap, elems):
                        dve_t = (elems + 140) / 0.96
                        act_t = (elems + 172) / 1.2
                        if eng_load[0] + dve_t <= eng_load[1] + act_t:
                            eng_load[0] += dve_t
                            nc.vector.tensor_copy(out_ap, in_ap)
                        else:
                            eng_load[1] += act_t
                            nc.scalar.copy(out_ap, in_ap)

                    for (plist, dst, plane) in ((ps_r, dst_r, 0), (ps_i, dst_i, 1)):
                        for (pt, c, sl) in plist:
                            if out_f32 is not None:
                                do_copy(out_f32[plane][:, sl], pt[:, :], CH)
                            elif plan is None:
                                do_copy(dst[:, sl], pt[:, :], CH)
                            else:
                                for (in_off, out_off, dims) in plan[c]:
                                    n_el = 1
                                    for (_, _, cnt) in dims:
                                        n_el *= cnt
                                    in_ap = sbuf_ap(pt, in_off, [(si, cnt) for (si, so, cnt) in dims])
                                    out_ap = sbuf_ap(dst, out_off, [(so, cnt) for (si, so, cnt) in dims])
                                    do_copy(out_ap, in_ap, n_el)
                    if round_done is not None:
                        round_done(rnd)

            def xbar(dst, src, order=None):
                # swap partition(7) <-> free-low-7, split into column eighths:
                # queue parallelism on HW, and the next window's first rounds
                # only depend on the early slices. `order` matches the slice
                # emission to the consumer's chunk order.
                for h in (order or range(16)):
                    sl = slice(h * 512, (h + 1) * 512)
                    out3 = dst[:, sl].rearrange("p (t j) -> p t j", t=4, j=128)
                    nc.sync.dma_start(out=out3, in_=src[:, sl], transpose=True)

            # windows 0..4 with transposes; split A2A; window 5
            cur_r, cur_i, oth_r, oth_i = Ar, Ai, Br, Bi
            for k in range(4):
                run_window(k, cur_r, cur_i, oth_r, oth_i)
                # before window 4 (q3-ordered chunks 0-3, 8-11, 4-7, 12-15),
                # emit the slices feeding its early rounds first
                t_order = [0,1,2,3,8,9,10,11,4,5,6,7,12,13,14,15] if k == 3 else None
                xbar(cur_r, oth_r, order=t_order)
                xbar(cur_i, oth_i, order=t_order)

            def emit_half(h):
                # blocks of half h are complete once V5's q3==h chunks copied
                for cp in range(NCORES):
                    sl = slice(cp * 1024 + h * 512, cp * 1024 + (h + 1) * 512)
                    nc.sync.dma_start(out=a2a_in_h[h][cp, 0], in_=oth_r[:, sl])
                    nc.sync.dma_start(out=a2a_in_h[h][cp, 1], in_=oth_i[:, sl])
                nc.gpsimd.collective_compute(
                    "AllToAll",
                    mybir.AluOpType.bypass,
                    replica_groups=[list(range(NCORES))],
                    ins=[a2a_in_h[h].ap().opt()],
                    outs=[a2a_out_h[h].ap().opt()],
                )

            # V5 source layout L_5 = [2,3,10,11,...]: chunk bit q3 has value 4
            v5_order = [c for c in range(NCH) if not (c & 4)] + \
                       [c for c in range(NCH) if (c & 4)]
            run_window(4, cur_r, cur_i, oth_r, oth_i,
                       chunk_order=v5_order,
                       round_done=lambda rnd: emit_half(rnd // 2) if rnd in (1, 3) else None)

            # compacted half tiles for window 5's input (keeps deps half-granular)
            G0r = fin_pool.tile([128, F // 2], BF16, tag="G0r")
            G0i = fin_pool.tile([128, F // 2], BF16, tag="G0i")
            G1r = fin_pool.tile([128, F // 2], BF16, tag="G1r")
            G1i = fin_pool.tile([128, F // 2], BF16, tag="G1i")
            G = [[G0r, G0i], [G1r, G1i]]
            for h in range(2):
                base = a2a_out_h[h].ap()
                for plane in range(2):
                    # [8, 2, 128, 512]: partition=(s, p_hi4), free=(p_lo3, f)
                    src = AP(base.tensor, base.offset + plane * 65536,
                             [[131072, 8], [4096, 16], [1, 4096]])
                    nc.sync.dma_start(out=G[h][plane][:, :], in_=src)

            # window 5 reads compacted halves: old chunk c -> G[c&1] col-block c>>1
            def v6_src(plane, c):
                return G[c & 1][plane], slice((c >> 1) * CH, ((c >> 1) + 1) * CH)

            v6_order = [c for c in range(NCH) if not (c & 1)] + \
                       [c for c in range(NCH) if (c & 1)]
            Fr = fin_pool.tile([128, F], BF16, tag="Fr")
            Fi = fin_pool.tile([128, F], BF16, tag="Fi")

            def v6_round_done(rnd):
                for cc in range(4):
                    c = v6_order[rnd * 4 + cc]
                    sl = slice(c * CH, (c + 1) * CH)
                    nc.sync.dma_start(out=out_ext[0, :, sl], in_=Fr[:, sl])
                    nc.sync.dma_start(out=out_ext[1, :, sl], in_=Fi[:, sl])

            run_window(5, None, None, None, None, out_f32=(Fr, Fi),
                       chunk_order=v6_order, src_of=v6_src,
                       round_done=v6_round_done)

    nc.compile()
    return nc


# ---------------------------------------------------------------------------
# host entry
# ---------------------------------------------------------------------------
_CACHED = {}


def _get_nc():
    if "nc" not in _CACHED:
        _CACHED["nc"] = build_bass()
    return _CACHED["nc"]


def kernel(state, gate_paras):
    from concourse.bass_utils import run_bass_kernel_spmd

    state = np.asarray(state)
    gate_paras = np.asarray(gate_paras)
    us = polar_unitaries(gate_paras)

    wmflat = np.zeros((128, 18 * 128), dtype=np.float64)
    for k, (pbits, gates) in enumerate(WINDOWS):
        W = window_matrix(pbits, gates, us)
        Wr, Wi = W.real, W.imag
        wmflat[:, (k * 3 + 0) * 128:(k * 3 + 1) * 128] = Wr.T
        wmflat[:, (k * 3 + 1) * 128:(k * 3 + 2) * 128] = -Wi.T
        wmflat[:, (k * 3 + 2) * 128:(k * 3 + 3) * 128] = Wi.T
    wm_bf = wmflat.astype(ml_dtypes.bfloat16)

    full = state.reshape(-1)
    in_maps = []
    for c in range(8):
        loc = full[c::8]
        in_maps.append({
            "xr": loc.real.astype(ml_dtypes.bfloat16).reshape(128, 8192),
            "xi": loc.imag.astype(ml_dtypes.bfloat16).reshape(128, 8192),
            "wm": wm_bf,
        })

    nc = _get_nc()
    res = run_bass_kernel_spmd(nc, in_maps, core_ids=list(range(8)))
    results = res.results

    out = np.zeros((2,) * N_QUBIT, dtype=np.complex64)
    natural = list(range(3, 23))
    perm = [FINAL_BITS.index(q) for q in natural]
    for c in range(8):
        o = results[c]["out"]
        arr = (o[0].astype(np.float32) + 1j * o[1].astype(np.float32)).astype(np.complex64)
        arr = arr.reshape((2,) * 20).transpose(perm)
        out[(c >> 2) & 1, (c >> 1) & 1, c & 1] = arr
    return out
